# revision 7
# baseline (speedup 1.0000x reference)
"""BiLinearInteraction Trainium2 kernel (8 NeuronCores, data-parallel over batch).

Reference computation (per pair p=(i,j) of F=26 fields, P=325 pairs):
    out[b, p*64:(p+1)*64] = (x[i, b, :] @ W[p]) * x[j, b, :]
Full shapes: x [26, 4096, 64] f32, W [325, 64, 64] f32 -> out [4096, 20800] f32.

Strategy (v3)
- Shard batch 4096 -> 8 x 512 (4 tiles of 128 rows/core), replicate W.
- HBM traffic ~28 MB/core: out 21.3MB bf16 write + reads ~6.6MB single-copy
  bf16. Even fields' matmul operands (lhsT xt, rhs w) sit in SBUF partitions
  0-63 and odd fields' in 64-127, so PE 2-row-group concurrency
  (tile_position row tiling) needs no duplicated HBM copies; consecutive
  fields' matmul pieces are emitted interleaved to pair the row groups.
- SWDGE descriptor generation costs ~850ns per dma_start serially on the Q7,
  so loads are consolidated to 9 issues: 7 per-chunk w loads (lo/hi packed in
  one [128, cmax] block, pad transferred on the narrow half) + tile-0
  (xn|xt) block + tiles-1-3 block. All input SBUF tiles are one-shot consts.
- Elementwise: measured rates ACT copy 0.833ns/el + 400ns/instr, DVE mul
  0.58ns/el from SBUF bf16 (2x mode, separate dst), 1.3ns/el from PSUM f32.
  Balanced split: the 12 biggest fields drain PSUM->bf16 cp tile on ACT then
  mul on DVE at 2x; the 13 smallest mul straight from PSUM. ~17.3/18.4us per
  tile on ACT/DVE.
- Output staged per (tile, chunk) and written as 7 contiguous bf16 DMAs/tile
  on the SP HWDGE ring; first chunk is one field so writes start early.
"""

import sys

sys.path.insert(0, "/opt/trn_rl_repo")

from itertools import combinations

import ml_dtypes
import numpy as np

import concourse.bass as bass
import concourse.mybir as mybir
from concourse import bacc
from concourse.tile import TileContext

F, D, B = 26, 64, 4096
NCORES = 8
BC = B // NCORES          # 512 batch rows per core
NT = BC // 128            # 4 batch tiles of 128 rows
NF = F - 1                # 25 left fields
PAIRS = list(combinations(range(F), 2))
N_PAIRS = [F - 1 - i for i in range(NF)]            # pairs with left field i
P_START = [sum(N_PAIRS[:i]) for i in range(NF)]     # first pair index of field i
P = sum(N_PAIRS)          # 325
OUT_COLS = P * D          # 20800

# column offset of field i inside the parity-packed w_lo / w_hi streams
WOFF = {}
_ol = _oh = 0
for _i in range(NF):
    if _i % 2 == 0:
        WOFF[_i] = _ol
        _ol += N_PAIRS[_i] * D
    else:
        WOFF[_i] = _oh
        _oh += N_PAIRS[_i] * D

# Output chunks: contiguous field ranges; first/last small for early writes
# and a short tail.
CHUNKS = [(0, 1), (1, 3), (3, 6), (6, 10), (10, 15), (15, 21), (21, 25)]
# per-chunk (first even field, lo cols), (first odd field, hi cols), padded max
CHUNK_LO, CHUNK_HI, CHUNK_MAX = [], [], []
for _f0, _f1 in CHUNKS:
    _ev = [i for i in range(_f0, _f1) if i % 2 == 0]
    _od = [i for i in range(_f0, _f1) if i % 2 == 1]
    _lc = sum(N_PAIRS[i] for i in _ev) * D
    _hc = sum(N_PAIRS[i] for i in _od) * D
    CHUNK_LO.append((_ev[0] if _ev else None, _lc))
    CHUNK_HI.append((_od[0] if _od else None, _hc))
    CHUNK_MAX.append(max(_lc, _hc))
W_PACK_COLS = sum(CHUNK_MAX)

N_DRAIN = 13              # fields 0..12 drained (ACT), 13..24 direct (DVE)
# PE row-group pairs (even field -> partitions 0-63, odd -> 64-127), chosen
# so each pair couples one ACT-drained (big) field with one DVE-direct
# (small) field: ACT and DVE stay loaded simultaneously through the tile
# instead of alternating drain-heavy then direct-heavy phases.
PAIR_ORDER = [(0, 13), (14, 1), (2, 15), (16, 3), (4, 17), (18, 5),
              (12, None), (6, 19), (20, 7), (8, 21), (22, 9), (10, 23),
              (24, 11)]
XT_BLK = (len([i for i in range(NF) if i % 2 == 0])) * 128   # 1664 lo cols
XX_TILE = F * D + XT_BLK  # 3328: [xn 1664 | xt 1664] per batch tile

F32 = mybir.dt.float32
BF16 = mybir.dt.bfloat16


def build_bass() -> bass.Bass:
    nc = bacc.Bacc()
    w = nc.declare_dram_parameter("w", [128, W_PACK_COLS], BF16, isOutput=False)
    xx0 = nc.declare_dram_parameter("xx0", [128, XX_TILE], BF16, isOutput=False)
    xx123 = nc.declare_dram_parameter(
        "xx123", [128, 3 * XX_TILE], BF16, isOutput=False)
    out = nc.declare_dram_parameter("out", [BC, OUT_COLS], BF16, isOutput=True)

    with TileContext(nc) as tc:
        with (
            tc.tile_pool(name="consts", bufs=1) as consts,
            tc.tile_pool(name="stage", bufs=2) as stage_pool,
            tc.tile_pool(name="cp_pool", bufs=3) as cp_pool,
            tc.tile_pool(name="psum", bufs=2, space="PSUM") as psum_pool,
        ):
            w_sb = [consts.tile([128, CHUNK_MAX[ci]], BF16,
                                tag=f"w{ci}", name=f"w{ci}")
                    for ci in range(len(CHUNKS))]
            xx0_sb = consts.tile([128, XX_TILE], BF16, tag="xx0", name="xx0")
            xx123_sb = consts.tile([128, 3 * XX_TILE], BF16,
                                   tag="xx123", name="xx123")

            # 9 SWDGE loads, just-in-time order: first chunk's weights and
            # tile-0 operands first; tiles 1-3 bulk last.
            _woff = [sum(CHUNK_MAX[:ci]) for ci in range(len(CHUNKS))]
            nc.gpsimd.dma_start(
                out=w_sb[0][:], in_=w[:, _woff[0]:_woff[0] + CHUNK_MAX[0]])
            nc.gpsimd.dma_start(out=xx0_sb[:], in_=xx0[:, :])
            for ci in range(1, len(CHUNKS)):
                nc.gpsimd.dma_start(
                    out=w_sb[ci][:], in_=w[:, _woff[ci]:_woff[ci] + CHUNK_MAX[ci]])
            nc.gpsimd.dma_start(out=xx123_sb[:], in_=xx123[:, :])

            field_chunk = {}
            for ci, (f0, f1) in enumerate(CHUNKS):
                for i in range(f0, f1):
                    field_chunk[i] = ci

            def xtile(t):
                return (xx0_sb, 0) if t == 0 else (xx123_sb, (t - 1) * XX_TILE)

            for t in range(NT):
                xsb, xbase = xtile(t)
                stage = {}
                remaining = {}
                for ci, (f0, f1) in enumerate(CHUNKS):
                    cols = sum(N_PAIRS[i] for i in range(f0, f1)) * D
                    stage[ci] = stage_pool.tile(
                        [128, cols], BF16, tag=f"st{ci}", name=f"st{t}_{ci}")
                    remaining[ci] = f1 - f0

                def mm_pieces(i):
                    npair = N_PAIRS[i]
                    cols = npair * D
                    g = i % 2
                    r0 = g * D
                    k = i // 2
                    ci = field_chunk[i]
                    lhsT = xsb[r0:r0 + D,
                               xbase + F * D + k * 128:xbase + F * D + (k + 1) * 128]
                    first = CHUNK_LO[ci][0] if g == 0 else CHUNK_HI[ci][0]
                    woff0 = WOFF[i] - WOFF[first]
                    ps = psum_pool.tile([128, cols], F32, tag="ps",
                                        name=f"ps{t}_{i}")
                    pieces = []
                    for s0 in range(0, cols, 512):
                        n = min(512, cols - s0)
                        pieces.append((ps[:, s0:s0 + n], lhsT,
                                       w_sb[ci][r0:r0 + D,
                                                woff0 + s0:woff0 + s0 + n]))
                    return ps, pieces

                def consume(i, ps):
                    npair = N_PAIRS[i]
                    cols = npair * D
                    ci = field_chunk[i]
                    st = stage[ci]
                    c0 = (P_START[i] - P_START[CHUNKS[ci][0]]) * D
                    dst = st[:, c0:c0 + cols]
                    xj = xsb[:, xbase + (i + 1) * D:xbase + (i + 1 + npair) * D]
                    if i < N_DRAIN:
                        cp = cp_pool.tile([128, cols], BF16, tag="cp",
                                          name=f"cp{t}_{i}")
                        nc.scalar.copy(out=cp[:], in_=ps[:])
                        nc.vector.tensor_mul(dst, cp[:], xj)
                    else:
                        nc.vector.tensor_mul(dst, ps[:], xj)
                    remaining[ci] -= 1
                    if remaining[ci] == 0:
                        f0, f1 = CHUNKS[ci]
                        cc0 = P_START[f0] * D
                        ccols = sum(N_PAIRS[j] for j in range(f0, f1)) * D
                        nc.sync.dma_start(
                            out=out[t * 128:(t + 1) * 128, cc0:cc0 + ccols],
                            in_=st[:])

                for fa, fb in PAIR_ORDER:
                    ps_a, pieces_a = mm_pieces(fa)
                    if fb is not None:
                        ps_b, pieces_b = mm_pieces(fb)
                    else:
                        ps_b, pieces_b = None, []
                    for pi in range(max(len(pieces_a), len(pieces_b))):
                        for pieces in (pieces_a, pieces_b):
                            if pi < len(pieces):
                                o, l, r = pieces[pi]
                                nc.tensor.matmul(o, l, r, start=True, stop=True)
                    if fb is None:
                        order = [fa]
                    else:
                        # direct (un-drained) field's mul first on DVE
                        order = [fa, fb] if fa >= N_DRAIN else [fb, fa]
                    for i in order:
                        consume(i, ps_a if i == fa else ps_b)
    nc.compile()
    return nc


def prep_inputs(x: np.ndarray, W: np.ndarray):
    """Full inputs -> per-core in_maps with pre-packed bf16 layouts."""
    x = np.ascontiguousarray(np.asarray(x, dtype=np.float32))
    W = np.ascontiguousarray(np.asarray(W, dtype=np.float32))
    # Pair-grouped weights wg[:, p*64+e] = W[p][:, e]; pack per chunk:
    # partitions 0-63 = even (lo) piece, 64-127 = odd (hi) piece, each
    # zero-padded to the chunk's max width.
    wg = W.transpose(1, 0, 2).reshape(D, OUT_COLS)
    wp = np.zeros((128, W_PACK_COLS), dtype=np.float32)
    col = 0
    for ci, (f0, f1) in enumerate(CHUNKS):
        lo = np.concatenate(
            [wg[:, P_START[i] * D:(P_START[i] + N_PAIRS[i]) * D]
             for i in range(f0, f1) if i % 2 == 0], axis=1)
        hi_parts = [wg[:, P_START[i] * D:(P_START[i] + N_PAIRS[i]) * D]
                    for i in range(f0, f1) if i % 2 == 1]
        wp[0:D, col:col + lo.shape[1]] = lo
        if hi_parts:
            hi = np.concatenate(hi_parts, axis=1)
            wp[D:2 * D, col:col + hi.shape[1]] = hi
        col += CHUNK_MAX[ci]
    wp = np.ascontiguousarray(wp.astype(ml_dtypes.bfloat16))

    EV = [i for i in range(NF) if i % 2 == 0]
    OD = [i for i in range(NF) if i % 2 == 1]
    in_maps = []
    for c in range(NCORES):
        xc = x[:, c * BC:(c + 1) * BC, :]                      # [26, 512, 64]
        xr = xc.reshape(F, NT, 128, D)
        xx = np.zeros((NT, 128, XX_TILE), dtype=np.float32)
        for t in range(NT):
            # xn block: [128, 26*64] batch-major field concat
            xx[t, :, :F * D] = xr[:, t].transpose(1, 0, 2).reshape(128, F * D)
            # xt block: [64, 13*128] per parity half (d-major lhsT layout)
            xtl = xr[EV, t].transpose(2, 0, 1).reshape(D, len(EV) * 128)
            xth = xr[OD, t].transpose(2, 0, 1).reshape(D, len(OD) * 128)
            xx[t, 0:D, F * D:F * D + xtl.shape[1]] = xtl
            xx[t, D:2 * D, F * D:F * D + xth.shape[1]] = xth
        xxb = xx.astype(ml_dtypes.bfloat16)
        in_maps.append({
            "w": wp,
            "xx0": np.ascontiguousarray(xxb[0]),
            "xx123": np.ascontiguousarray(
                xxb[1:].transpose(1, 0, 2).reshape(128, 3 * XX_TILE)),
        })
    return in_maps


_CACHED_NC = None


def kernel(x: np.ndarray, W: np.ndarray) -> np.ndarray:
    global _CACHED_NC
    from concourse.bass_utils import run_bass_kernel_spmd

    if _CACHED_NC is None:
        _CACHED_NC = build_bass()
    in_maps = prep_inputs(x, W)
    res = run_bass_kernel_spmd(_CACHED_NC, in_maps, list(range(NCORES)))
    shards = [
        np.asarray(res.results[c]["out"]).astype(np.float32) for c in range(NCORES)
    ]
    return np.concatenate(shards, axis=0)
